# revision 7
# baseline (speedup 1.0000x reference)
"""Causal self-attention (B=4, T=2048, C=1024, H=16) on 8 TRN2 NeuronCores.

Sharding: batch x head-half. Core c handles batch c//2 and heads
8*(c%2) .. 8*(c%2)+8. Each core computes qkv for its 8 heads (w_attn column
shard), full causal attention for those heads, and a partial c_proj product
(w_proj row shard). The host sums the two partials per batch (the 2-way
all-reduce of the TP scheme) and adds b_proj.

Per-core program (S^T formulation: softmax reduces on the free axis via an
appended ones-column in the PV weights; no max-subtraction, scores are
bounded ~N(0, 0.41^2)):
  phase 1: qkv^T = wqkv^T @ x^T (bf16 matmuls, f32 psum); ACT engine applies
    bias and casts psum->sbuf bf16 (it is otherwise idle here).
  phase 1.5: v^T tiles are PE-transposed pairwise ([128,128] covers 2 heads)
    into v65 tiles [128, 65] bf16 whose col 64 is ones (denominator trick).
  phase 2: for tqb (512-query blocks) DESCENDING, for each head: S^T tiles
    k_i^T-stationary x q^T-moving into [128,1024] psum groups (2 key tiles),
    one ACT exp per group -> P bf16; causal diag tiles masked by a DVE
    triangular multiply; PV accumulates [65,512] per query block; the
    denominator row is inverted with reciprocal_approx_fast (DVE),
    partition-broadcast on GpSimd, and multiplied into attn out (DVE).
  phase 3 (interleaved per tqb): proj tiles y = attn_pack @ wp accumulate
    over the 4 head-pairs in psum and DMA straight from psum to DRAM.
"""

import sys

for _p in (
    "/opt/trn_rl_repo",
    "/root/.axon_site/_ro/trn_rl_repo",
):
    if _p not in sys.path:
        sys.path.append(_p)

import numpy as np
import ml_dtypes
import concourse.bacc as bacc
import concourse.mybir as mybir
import concourse.tile as tile
from concourse.bass_utils import run_bass_kernel_spmd
from concourse.masks import make_identity, make_upper_triangular

B, T, C, H = 4, 2048, 1024, 16
HS = C // H           # 64
NCORES = 8
HPC = H // 2          # heads per core = 8
NPAIR = HPC // 2      # head pairs per core = 4
MQKV = 3 * HPC * HS   # 1536 qkv columns per core
KT = C // 128         # 8 contraction tiles
NBB = T // 512        # 4 query/token blocks
TTK = T // 128        # 16 key tiles per sequence
NT = T // 128         # 16 proj row tiles

f32 = mybir.dt.float32
bf16 = mybir.dt.bfloat16
EXPF = mybir.ActivationFunctionType.Exp
IDENTF = mybir.ActivationFunctionType.Identity
PSUM = "PSUM"


def build_nc():
    nc = bacc.Bacc("TRN2", target_bir_lowering=False, debug=False, num_devices=NCORES)
    xT_d = nc.dram_tensor("xT", [C, T], bf16, kind="ExternalInput")
    wqkv_d = nc.dram_tensor("wqkv", [C, MQKV], bf16, kind="ExternalInput")
    bqkv_d = nc.dram_tensor("bqkv", [MQKV], f32, kind="ExternalInput")
    wp_d = nc.dram_tensor("wp", [HPC * HS, C], bf16, kind="ExternalInput")
    y_d = nc.dram_tensor("y", [T, C], f32, kind="ExternalOutput")

    w_src = wqkv_d.ap().rearrange("(k p) m -> p k m", p=128)
    xT_src = xT_d.ap().rearrange("(k p) n -> p k n", p=128)
    wp_src = wp_d.ap().rearrange("(k p) n -> p k n", p=128)

    with tile.TileContext(nc) as tc:
        with tc.tile_pool(name="const", bufs=1) as const, tc.tile_pool(
            name="qkvp", bufs=1
        ) as qkvp, tc.tile_pool(name="v65", bufs=1) as v65p, tc.tile_pool(
            name="pp", bufs=4
        ) as ppool, tc.tile_pool(name="sml", bufs=4) as smlp, tc.tile_pool(
            name="ps_st", bufs=2, space=PSUM
        ) as stp, tc.tile_pool(name="ps_po", bufs=2, space=PSUM) as pop, tc.tile_pool(
            name="ps_mm", bufs=2, space=PSUM
        ) as mmp:
            # ---- constants ----
            ident = const.tile([128, 128], f32)
            make_identity(nc, ident)
            ident_b = const.tile([128, 128], bf16)
            nc.vector.tensor_copy(ident_b[:], ident[:])
            tri = const.tile([128, 128], f32)
            make_upper_triangular(nc, tri, val=1.0, diag=True)  # 1 where part<=free
            tri_b = const.tile([128, 128], bf16)
            nc.vector.tensor_copy(tri_b[:], tri[:])
            bias_sb = const.tile([128, MQKV // 128], f32)
            nc.sync.dma_start(bias_sb[:], bqkv_d.ap().rearrange("(m p) -> p m", p=128))
            w_sb = const.tile([128, KT, MQKV], bf16)
            nc.sync.dma_start(w_sb[:], w_src)
            wp_sb = const.tile([128, NPAIR, C], bf16)
            nc.sync.dma_start(wp_sb[:], wp_src)
            xt = const.tile([128, KT, T], bf16)
            for nb in range(NBB):
                nc.sync.dma_start(
                    xt[:, :, 512 * nb : 512 * (nb + 1)],
                    xT_src[:, :, 512 * nb : 512 * (nb + 1)],
                )

            # v65[pair]: [128 keys, key-tile i, head-in-pair hh, 64 dims + ones]
            v65 = [
                const.tile([128, TTK, 2, HS + 1], bf16, name=f"v65_{p}")
                for p in range(NPAIR)
            ]
            for p in range(NPAIR):
                nc.vector.memset(v65[p][:, :, :, HS : HS + 1], 1.0)

            # attn_pack[pair]: [128 = 2*HS head dims, T] bf16 proj input
            attn_pack = [
                const.tile([128, T], bf16, name=f"attn_pack_{p}") for p in range(NPAIR)
            ]

            # ---- phase 1: qkv^T (m-tile m covers 128 qkv columns) ----
            qkvT = [
                qkvp.tile([128, T], bf16, tag=f"qkvT{m}", name=f"qkvT{m}")
                for m in range(MQKV // 128)
            ]
            for nb in range(NBB):
                for m in range(MQKV // 128):
                    pq = mmp.tile([128, 512], f32, tag="mm")
                    for k in range(KT):
                        nc.tensor.matmul(
                            pq[:],
                            w_sb[:, k, 128 * m : 128 * (m + 1)],
                            xt[:, k, 512 * nb : 512 * (nb + 1)],
                            start=(k == 0),
                            stop=(k == KT - 1),
                        )
                    # bias + cast on ACT (idle during this phase)
                    nc.scalar.activation(
                        qkvT[m][:, 512 * nb : 512 * (nb + 1)],
                        pq[:],
                        IDENTF,
                        bias=bias_sb[:, m : m + 1],
                    )

            # ---- phase 1.5: v transposes, 2 heads per [128,128] tile ----
            for p in range(NPAIR):
                vm = qkvT[2 * NPAIR + p]  # v m-tile of this pair
                for i in range(TTK):
                    pt = mmp.tile([128, 128], bf16, tag="mm", name="vt")
                    nc.tensor.transpose(
                        pt[:], vm[:, 128 * i : 128 * (i + 1)], ident_b[:]
                    )
                    # psum [128 keys, 2*64 dims] -> v65[:, i, hh, 0:64]
                    nc.vector.tensor_copy(v65[p][:, i, :, 0:HS], pt[:])

            # ---- phase 2+3: attention (tqb descending) + proj per tqb ----
            def emit_proj(tqb):
                for t in range(4 * tqb, 4 * tqb + 4):
                    ty = smlp.tile([128, C], f32, tag="y", bufs=3, name="ty")
                    for n in range(2):
                        py = mmp.tile([128, 512], f32, tag="mm", name="py")
                        for p in range(NPAIR):
                            nc.tensor.matmul(
                                py[:],
                                attn_pack[p][:, 128 * t : 128 * (t + 1)],
                                wp_sb[:, p, 512 * n : 512 * (n + 1)],
                                start=(p == 0),
                                stop=(p == NPAIR - 1),
                            )
                        nc.vector.tensor_copy(ty[:, 512 * n : 512 * (n + 1)], py[:])
                    nc.scalar.dma_start(y_d[128 * t : 128 * (t + 1), :], ty[:])

            pending_proj = None
            for tqbi in range(NBB):
                tqb = NBB - 1 - tqbi
                q0 = 512 * tqb
                ntk = 4 * (tqb + 1)
                for p in range(NPAIR):
                    qm = qkvT[p]
                    km = qkvT[NPAIR + p]
                    for hh in range(2):
                        hp = HS * hh
                        po = pop.tile([HS + 1, 512], f32, tag="po")
                        for ig in range(ntk // 2):
                            st = stp.tile([128, 1024], f32, tag="st")
                            ptile = ppool.tile([128, 1024], bf16, tag="p", name="pt")
                            # pack tile u=1 right after u=0's end so one exp
                            # call covers [vf0 : 1024-vf1] contiguously
                            i0, i1 = 2 * ig, 2 * ig + 1
                            vf0 = max(0, 128 * i0 - q0)
                            vf1 = max(0, 128 * i1 - q0)
                            nc.tensor.matmul(
                                st[:, vf0:512],
                                km[hp : hp + HS, 128 * i0 : 128 * (i0 + 1)],
                                qm[hp : hp + HS, q0 + vf0 : q0 + 512],
                                start=True,
                                stop=True,
                            )
                            nc.tensor.matmul(
                                st[:, 512 : 1024 - vf1],
                                km[hp : hp + HS, 128 * i1 : 128 * (i1 + 1)],
                                qm[hp : hp + HS, q0 + vf1 : q0 + 512],
                                start=True,
                                stop=True,
                            )
                            nc.scalar.activation(
                                ptile[:, vf0 : 1024 - vf1],
                                st[:, vf0 : 1024 - vf1],
                                EXPF,
                            )
                            if 128 * i0 >= q0:  # diagonal tiles: causal mask
                                nc.gpsimd.tensor_mul(
                                    ptile[:, vf0 : vf0 + 128],
                                    ptile[:, vf0 : vf0 + 128],
                                    tri_b[:],
                                )
                            if 128 * i1 >= q0:
                                nc.gpsimd.tensor_mul(
                                    ptile[:, 512:640],
                                    ptile[:, 512:640],
                                    tri_b[:],
                                )
                            nc.tensor.matmul(
                                po[:, vf0:512],
                                v65[p][:, i0, hh, :],
                                ptile[:, vf0:512],
                                start=(i0 == 0),
                                stop=False,
                            )
                            nc.tensor.matmul(
                                po[:, vf1:512],
                                v65[p][:, i1, hh, :],
                                ptile[:, 512 : 1024 - vf1],
                                start=False,
                                stop=(i1 == ntk - 1),
                            )
                        # custom-DVE ops ignore the input partition base, so
                        # hop the denominator row to partition 0 first
                        zrow = smlp.tile([1, 512], f32, tag="zrow")
                        nc.vector.tensor_copy(zrow[:], po[HS : HS + 1, :])
                        recip = smlp.tile([1, 512], f32, tag="rcp")
                        nc.vector.reciprocal_approx_fast(recip[:], zrow[:])
                        rb = smlp.tile([HS, 512], f32, tag="rb")
                        nc.gpsimd.partition_broadcast(rb[:], recip[:])
                        if hh == 0:
                            nc.vector.tensor_mul(
                                attn_pack[p][0:HS, q0 : q0 + 512],
                                po[0:HS, :],
                                rb[:],
                            )
                        else:
                            a1 = smlp.tile([HS, 512], bf16, tag="a1")
                            nc.vector.tensor_mul(a1[:], po[0:HS, :], rb[:])
                            nc.sync.dma_start(
                                attn_pack[p][HS:128, q0 : q0 + 512], a1[:]
                            )
                    # interleave the previous tqb's proj behind pair 0 so the
                    # PE's proj work doesn't starve ACT at the tqb boundary
                    if p == 0 and pending_proj is not None:
                        emit_proj(pending_proj)
                        pending_proj = None
                pending_proj = tqb
            emit_proj(pending_proj)

    nc.compile()
    return nc


_NC_CACHE = None


def _get_nc():
    global _NC_CACHE
    if _NC_CACHE is None:
        _NC_CACHE = build_nc()
    return _NC_CACHE


def make_in_maps(x, w_attn, b_attn, w_proj):
    x = np.asarray(x, np.float32).reshape(B, T, C)
    w_attn = np.asarray(w_attn, np.float32)
    b_attn = np.asarray(b_attn, np.float32)
    w_proj = np.asarray(w_proj, np.float32)
    scale = 1.0 / np.sqrt(HS)
    in_maps = []
    for c in range(NCORES):
        b, j = divmod(c, 2)
        cs = slice(HS * HPC * j, HS * HPC * (j + 1))
        wq = w_attn[:, 0 * C :][:, cs] * scale
        wk = w_attn[:, 1 * C : 2 * C][:, cs]
        wv = w_attn[:, 2 * C : 3 * C][:, cs]
        wqkv = np.concatenate([wq, wk, wv], axis=1)
        bq = b_attn[0 * C :][cs] * scale
        bk = b_attn[1 * C : 2 * C][cs]
        bv = b_attn[2 * C : 3 * C][cs]
        bqkv = np.ascontiguousarray(np.concatenate([bq, bk, bv]))
        in_maps.append(
            {
                "xT": np.ascontiguousarray(x[b].T).astype(ml_dtypes.bfloat16),
                "wqkv": wqkv.astype(ml_dtypes.bfloat16),
                "bqkv": bqkv,
                "wp": w_proj[cs, :].astype(ml_dtypes.bfloat16),
            }
        )
    return in_maps


def run_on_device(in_maps, **kwargs):
    nc = _get_nc()
    return run_bass_kernel_spmd(nc, in_maps, core_ids=list(range(NCORES)), **kwargs)


def kernel(x, w_attn, b_attn, w_proj, b_proj):
    in_maps = make_in_maps(x, w_attn, b_attn, w_proj)
    res = run_on_device(in_maps)
    b_proj = np.asarray(b_proj, np.float32)
    y = np.empty((B, T, C), np.float32)
    for b in range(B):
        y[b] = res.results[2 * b]["y"]
        y[b] += res.results[2 * b + 1]["y"]
        y[b] += b_proj
    return y


if __name__ == "__main__":
    rng = np.random.default_rng(0)
    x = rng.standard_normal((B, T, C)).astype(np.float32)
    w_attn = (rng.standard_normal((C, 3 * C)) * 0.02).astype(np.float32)
    b_attn = np.zeros(3 * C, np.float32)
    w_proj = (rng.standard_normal((C, C)) * 0.02).astype(np.float32)
    b_proj = np.zeros(C, np.float32)
    y = kernel(x, w_attn, b_attn, w_proj, b_proj)
    print("out", y.shape, y.dtype, y[0, 0, :4])


# revision 8
# speedup vs baseline: 1.8890x; 1.8890x over previous
"""Causal self-attention (B=4, T=2048, C=1024, H=16) on 8 TRN2 NeuronCores.

Sharding: batch x head-half. Core c handles batch c//2 and heads
8*(c%2) .. 8*(c%2)+8. Each core computes qkv for its 8 heads (w_attn column
shard), full causal attention for those heads, and a partial c_proj product
(w_proj row shard). The host sums the two partials per batch (the 2-way
all-reduce of the TP scheme) and adds b_proj.

Per-core program (S^T formulation: softmax reduces on the free axis via an
appended ones-column in the PV weights; no max-subtraction, scores are
bounded ~N(0, 0.41^2)):
  phase 1: qkv^T = wqkv^T @ x^T (bf16 matmuls, f32 psum); ACT engine applies
    bias and casts psum->sbuf bf16 (it is otherwise idle here).
  phase 1.5: v^T tiles are PE-transposed pairwise ([128,128] covers 2 heads)
    into v65 tiles [128, 65] bf16 whose col 64 is ones (denominator trick).
  phase 2: for tqb (512-query blocks) DESCENDING, for each head: S^T tiles
    k_i^T-stationary x q^T-moving into [128,1024] psum groups (2 key tiles),
    one ACT exp per group -> P bf16; causal diag tiles masked by a DVE
    triangular multiply; PV accumulates [65,512] per query block; the
    denominator row is inverted with reciprocal_approx_fast (DVE),
    partition-broadcast on GpSimd, and multiplied into attn out (DVE).
  phase 3 (interleaved per tqb): proj tiles y = attn_pack @ wp accumulate
    over the 4 head-pairs in psum and DMA straight from psum to DRAM.
"""

import sys

for _p in (
    "/opt/trn_rl_repo",
    "/root/.axon_site/_ro/trn_rl_repo",
):
    if _p not in sys.path:
        sys.path.append(_p)

import numpy as np
import ml_dtypes
import concourse.bacc as bacc
import concourse.mybir as mybir
import concourse.tile as tile
from concourse.bass_utils import run_bass_kernel_spmd
from concourse.masks import make_identity, make_upper_triangular

B, T, C, H = 4, 2048, 1024, 16
HS = C // H           # 64
NCORES = 8
HPC = H // 2          # heads per core = 8
NPAIR = HPC // 2      # head pairs per core = 4
MQKV = 3 * HPC * HS   # 1536 qkv columns per core
KT = C // 128         # 8 contraction tiles
NBB = T // 512        # 4 query/token blocks
TTK = T // 128        # 16 key tiles per sequence
NT = T // 128         # 16 proj row tiles

f32 = mybir.dt.float32
bf16 = mybir.dt.bfloat16
EXPF = mybir.ActivationFunctionType.Exp
IDENTF = mybir.ActivationFunctionType.Identity
PSUM = "PSUM"


def build_nc():
    nc = bacc.Bacc("TRN2", target_bir_lowering=False, debug=False, num_devices=NCORES)
    xT_d = nc.dram_tensor("xT", [C, T], bf16, kind="ExternalInput")
    wqkv_d = nc.dram_tensor("wqkv", [C, MQKV], bf16, kind="ExternalInput")
    bqkv_d = nc.dram_tensor("bqkv", [MQKV], f32, kind="ExternalInput")
    wp_d = nc.dram_tensor("wp", [HPC * HS, C], bf16, kind="ExternalInput")
    y_d = nc.dram_tensor("y", [T, C], f32, kind="ExternalOutput")

    w_src = wqkv_d.ap().rearrange("(k p) m -> p k m", p=128)
    xT_src = xT_d.ap().rearrange("(k p) n -> p k n", p=128)
    wp_src = wp_d.ap().rearrange("(k p) n -> p k n", p=128)

    with tile.TileContext(nc) as tc:
        with tc.tile_pool(name="const", bufs=1) as const, tc.tile_pool(
            name="qkvp", bufs=1
        ) as qkvp, tc.tile_pool(name="v65", bufs=1) as v65p, tc.tile_pool(
            name="pp", bufs=4
        ) as ppool, tc.tile_pool(name="sml", bufs=4) as smlp, tc.tile_pool(
            name="ps_st", bufs=2, space=PSUM
        ) as stp, tc.tile_pool(name="ps_po", bufs=2, space=PSUM) as pop, tc.tile_pool(
            name="ps_mm", bufs=2, space=PSUM
        ) as mmp:
            # ---- constants ----
            ident = const.tile([128, 128], f32)
            make_identity(nc, ident)
            ident_b = const.tile([128, 128], bf16)
            nc.vector.tensor_copy(ident_b[:], ident[:])
            tri = const.tile([128, 128], f32)
            make_upper_triangular(nc, tri, val=1.0, diag=True)  # 1 where part<=free
            tri_b = const.tile([128, 128], bf16)
            nc.vector.tensor_copy(tri_b[:], tri[:])
            bias_sb = const.tile([128, MQKV // 128], f32)
            nc.sync.dma_start(bias_sb[:], bqkv_d.ap().rearrange("(m p) -> p m", p=128))
            w_sb = const.tile([128, KT, MQKV], bf16)
            nc.sync.dma_start(w_sb[:], w_src)
            wp_sb = const.tile([128, NPAIR, C], bf16)
            nc.sync.dma_start(wp_sb[:], wp_src)
            xt = const.tile([128, KT, T], bf16)
            for nb in range(NBB):
                nc.sync.dma_start(
                    xt[:, :, 512 * nb : 512 * (nb + 1)],
                    xT_src[:, :, 512 * nb : 512 * (nb + 1)],
                )

            # v65[pair]: [128 keys, key-tile i, head-in-pair hh, 64 dims + ones]
            v65 = [
                const.tile([128, TTK, 2, HS + 1], bf16, name=f"v65_{p}")
                for p in range(NPAIR)
            ]
            for p in range(NPAIR):
                nc.vector.memset(v65[p][:, :, :, HS : HS + 1], 1.0)

            # attn_pack[pair]: [128 = 2*HS head dims, T] bf16 proj input
            attn_pack = [
                const.tile([128, T], bf16, name=f"attn_pack_{p}") for p in range(NPAIR)
            ]

            # ---- phase 1: qkv^T (m-tile m covers 128 qkv columns) ----
            qkvT = [
                qkvp.tile([128, T], bf16, tag=f"qkvT{m}", name=f"qkvT{m}")
                for m in range(MQKV // 128)
            ]
            for nb in range(NBB):
                for m in range(MQKV // 128):
                    pq = mmp.tile([128, 512], f32, tag="mm")
                    for k in range(KT):
                        nc.tensor.matmul(
                            pq[:],
                            w_sb[:, k, 128 * m : 128 * (m + 1)],
                            xt[:, k, 512 * nb : 512 * (nb + 1)],
                            start=(k == 0),
                            stop=(k == KT - 1),
                        )
                    # bias + cast on ACT (idle during this phase)
                    nc.scalar.activation(
                        qkvT[m][:, 512 * nb : 512 * (nb + 1)],
                        pq[:],
                        IDENTF,
                        bias=bias_sb[:, m : m + 1],
                    )

            # ---- phase 1.5: v transposes, 2 heads per [128,128] tile ----
            for p in range(NPAIR):
                vm = qkvT[2 * NPAIR + p]  # v m-tile of this pair
                for i in range(TTK):
                    pt = mmp.tile([128, 128], bf16, tag="mm", name="vt")
                    nc.tensor.transpose(
                        pt[:], vm[:, 128 * i : 128 * (i + 1)], ident_b[:]
                    )
                    # psum [128 keys, 2*64 dims] -> v65[:, i, hh, 0:64]
                    nc.vector.tensor_copy(v65[p][:, i, :, 0:HS], pt[:])

            # ---- phase 2+3: attention (tqb descending) + proj per tqb ----
            def emit_proj(tqb):
                for t in range(4 * tqb, 4 * tqb + 4):
                    ty = smlp.tile([128, C], f32, tag="y", bufs=3, name="ty")
                    for n in range(2):
                        py = mmp.tile([128, 512], f32, tag="mm", name="py")
                        for p in range(NPAIR):
                            nc.tensor.matmul(
                                py[:],
                                attn_pack[p][:, 128 * t : 128 * (t + 1)],
                                wp_sb[:, p, 512 * n : 512 * (n + 1)],
                                start=(p == 0),
                                stop=(p == NPAIR - 1),
                            )
                        nc.vector.tensor_copy(ty[:, 512 * n : 512 * (n + 1)], py[:])
                    nc.scalar.dma_start(y_d[128 * t : 128 * (t + 1), :], ty[:])

            pending_proj = None
            for tqbi in range(NBB):
                tqb = NBB - 1 - tqbi
                q0 = 512 * tqb
                ntk = 4 * (tqb + 1)
                for p in range(NPAIR):
                    qm = qkvT[p]
                    km = qkvT[NPAIR + p]
                    for hh in range(2):
                        hp = HS * hh
                        po = pop.tile([HS + 1, 512], f32, tag="po")
                        for ig in range(ntk // 2):
                            st = stp.tile([128, 1024], f32, tag="st")
                            ptile = ppool.tile([128, 1024], bf16, tag="p", name="pt")
                            # pack tile u=1 right after u=0's end so one exp
                            # call covers [vf0 : 1024-vf1] contiguously
                            i0, i1 = 2 * ig, 2 * ig + 1
                            vf0 = max(0, 128 * i0 - q0)
                            vf1 = max(0, 128 * i1 - q0)
                            nc.tensor.matmul(
                                st[:, vf0:512],
                                km[hp : hp + HS, 128 * i0 : 128 * (i0 + 1)],
                                qm[hp : hp + HS, q0 + vf0 : q0 + 512],
                                start=True,
                                stop=True,
                            )
                            nc.tensor.matmul(
                                st[:, 512 : 1024 - vf1],
                                km[hp : hp + HS, 128 * i1 : 128 * (i1 + 1)],
                                qm[hp : hp + HS, q0 + vf1 : q0 + 512],
                                start=True,
                                stop=True,
                            )
                            nc.scalar.activation(
                                ptile[:, vf0 : 1024 - vf1],
                                st[:, vf0 : 1024 - vf1],
                                EXPF,
                            )
                            if 128 * i0 >= q0:  # diagonal tiles: causal mask
                                nc.vector.tensor_mul(
                                    ptile[:, vf0 : vf0 + 128],
                                    ptile[:, vf0 : vf0 + 128],
                                    tri_b[:],
                                )
                            if 128 * i1 >= q0:
                                nc.vector.tensor_mul(
                                    ptile[:, 512:640],
                                    ptile[:, 512:640],
                                    tri_b[:],
                                )
                            nc.tensor.matmul(
                                po[:, vf0:512],
                                v65[p][:, i0, hh, :],
                                ptile[:, vf0:512],
                                start=(i0 == 0),
                                stop=False,
                            )
                            nc.tensor.matmul(
                                po[:, vf1:512],
                                v65[p][:, i1, hh, :],
                                ptile[:, 512 : 1024 - vf1],
                                start=False,
                                stop=(i1 == ntk - 1),
                            )
                        # custom-DVE ops ignore the input partition base, so
                        # hop the denominator row to partition 0 first
                        zrow = smlp.tile([1, 512], f32, tag="zrow")
                        nc.vector.tensor_copy(zrow[:], po[HS : HS + 1, :])
                        recip = smlp.tile([1, 512], f32, tag="rcp")
                        nc.vector.reciprocal_approx_fast(recip[:], zrow[:])
                        rb = smlp.tile([HS, 512], f32, tag="rb")
                        nc.gpsimd.partition_broadcast(rb[:], recip[:])
                        if hh == 0:
                            nc.vector.tensor_mul(
                                attn_pack[p][0:HS, q0 : q0 + 512],
                                po[0:HS, :],
                                rb[:],
                            )
                        else:
                            a1 = smlp.tile([HS, 512], bf16, tag="a1")
                            nc.vector.tensor_mul(a1[:], po[0:HS, :], rb[:])
                            nc.sync.dma_start(
                                attn_pack[p][HS:128, q0 : q0 + 512], a1[:]
                            )
                    # interleave the previous tqb's proj behind pair 0 so the
                    # PE's proj work doesn't starve ACT at the tqb boundary
                    if p == 0 and pending_proj is not None:
                        emit_proj(pending_proj)
                        pending_proj = None
                pending_proj = tqb
            emit_proj(pending_proj)

    nc.compile()
    return nc


_NC_CACHE = None


def _get_nc():
    global _NC_CACHE
    if _NC_CACHE is None:
        _NC_CACHE = build_nc()
    return _NC_CACHE


def make_in_maps(x, w_attn, b_attn, w_proj):
    x = np.asarray(x, np.float32).reshape(B, T, C)
    w_attn = np.asarray(w_attn, np.float32)
    b_attn = np.asarray(b_attn, np.float32)
    w_proj = np.asarray(w_proj, np.float32)
    scale = 1.0 / np.sqrt(HS)
    in_maps = []
    for c in range(NCORES):
        b, j = divmod(c, 2)
        cs = slice(HS * HPC * j, HS * HPC * (j + 1))
        wq = w_attn[:, 0 * C :][:, cs] * scale
        wk = w_attn[:, 1 * C : 2 * C][:, cs]
        wv = w_attn[:, 2 * C : 3 * C][:, cs]
        wqkv = np.concatenate([wq, wk, wv], axis=1)
        bq = b_attn[0 * C :][cs] * scale
        bk = b_attn[1 * C : 2 * C][cs]
        bv = b_attn[2 * C : 3 * C][cs]
        bqkv = np.ascontiguousarray(np.concatenate([bq, bk, bv]))
        in_maps.append(
            {
                "xT": np.ascontiguousarray(x[b].T).astype(ml_dtypes.bfloat16),
                "wqkv": wqkv.astype(ml_dtypes.bfloat16),
                "bqkv": bqkv,
                "wp": w_proj[cs, :].astype(ml_dtypes.bfloat16),
            }
        )
    return in_maps


def run_on_device(in_maps, **kwargs):
    nc = _get_nc()
    return run_bass_kernel_spmd(nc, in_maps, core_ids=list(range(NCORES)), **kwargs)


def kernel(x, w_attn, b_attn, w_proj, b_proj):
    in_maps = make_in_maps(x, w_attn, b_attn, w_proj)
    res = run_on_device(in_maps)
    b_proj = np.asarray(b_proj, np.float32)
    y = np.empty((B, T, C), np.float32)
    for b in range(B):
        y[b] = res.results[2 * b]["y"]
        y[b] += res.results[2 * b + 1]["y"]
        y[b] += b_proj
    return y


if __name__ == "__main__":
    rng = np.random.default_rng(0)
    x = rng.standard_normal((B, T, C)).astype(np.float32)
    w_attn = (rng.standard_normal((C, 3 * C)) * 0.02).astype(np.float32)
    b_attn = np.zeros(3 * C, np.float32)
    w_proj = (rng.standard_normal((C, C)) * 0.02).astype(np.float32)
    b_proj = np.zeros(C, np.float32)
    y = kernel(x, w_attn, b_attn, w_proj, b_proj)
    print("out", y.shape, y.dtype, y[0, 0, :4])


# revision 13
# speedup vs baseline: 2.0270x; 1.0731x over previous
"""Causal self-attention (B=4, T=2048, C=1024, H=16) on 8 TRN2 NeuronCores.

Sharding: batch x head-half. Core c handles batch c//2 and heads
8*(c%2) .. 8*(c%2)+8. Each core computes qkv for its 8 heads (w_attn column
shard), full causal attention for those heads, and a partial c_proj product
(w_proj row shard). The host sums the two partials per batch (the 2-way
all-reduce of the TP scheme) and adds b_proj.

Per-core program (S^T formulation: softmax reduces on the free axis via an
appended ones-column in the PV weights; no max-subtraction, scores are
bounded ~N(0, 0.41^2)):
  phase 1: qkv^T = wqkv^T @ x^T (bf16 matmuls, f32 psum); ACT engine applies
    bias and casts psum->sbuf bf16 (it is otherwise idle here).
  phase 1.5: v^T tiles are PE-transposed pairwise ([128,128] covers 2 heads)
    into v65 tiles [128, 65] bf16 whose col 64 is ones (denominator trick).
  phase 2: for tqb (512-query blocks) DESCENDING, for each head: S^T tiles
    k_i^T-stationary x q^T-moving into [128,1024] psum groups (2 key tiles),
    one ACT exp per group -> P bf16; causal diag tiles masked by a DVE
    triangular multiply; PV accumulates [65,512] per query block; the
    denominator row is inverted with reciprocal_approx_fast (DVE),
    partition-broadcast on GpSimd, and multiplied into attn out (DVE).
  phase 3 (interleaved per tqb): proj tiles y = attn_pack @ wp accumulate
    over the 4 head-pairs in psum and DMA straight from psum to DRAM.
"""

import sys

for _p in (
    "/opt/trn_rl_repo",
    "/root/.axon_site/_ro/trn_rl_repo",
):
    if _p not in sys.path:
        sys.path.append(_p)

import numpy as np
import ml_dtypes
import concourse.bacc as bacc
import concourse.mybir as mybir
import concourse.tile as tile
from concourse.bass_utils import run_bass_kernel_spmd
from concourse.masks import make_identity, make_upper_triangular

B, T, C, H = 4, 2048, 1024, 16
HS = C // H           # 64
NCORES = 8
HPC = H // 2          # heads per core = 8
NPAIR = HPC // 2      # head pairs per core = 4
MQKV = 3 * HPC * HS   # 1536 qkv columns per core
KT = C // 128         # 8 contraction tiles
NBB = T // 512        # 4 query/token blocks
TTK = T // 128        # 16 key tiles per sequence
NT = T // 128         # 16 proj row tiles

f32 = mybir.dt.float32
bf16 = mybir.dt.bfloat16
EXPF = mybir.ActivationFunctionType.Exp
IDENTF = mybir.ActivationFunctionType.Identity
PSUM = "PSUM"


def build_nc():
    nc = bacc.Bacc("TRN2", target_bir_lowering=False, debug=False, num_devices=NCORES)
    xT_d = nc.dram_tensor("xT", [C, T], bf16, kind="ExternalInput")
    wqkv_d = nc.dram_tensor("wqkv", [C, MQKV], bf16, kind="ExternalInput")
    bqkv_d = nc.dram_tensor("bqkv", [MQKV], f32, kind="ExternalInput")
    wp_d = nc.dram_tensor("wp", [HPC * HS, C], bf16, kind="ExternalInput")
    y_d = nc.dram_tensor("y", [T, C], f32, kind="ExternalOutput")

    w_src = wqkv_d.ap().rearrange("(k p) m -> p k m", p=128)
    xT_src = xT_d.ap().rearrange("(k p) n -> p k n", p=128)
    wp_src = wp_d.ap().rearrange("(k p) n -> p k n", p=128)

    with tile.TileContext(nc) as tc:
        with tc.tile_pool(name="const", bufs=1) as const, tc.tile_pool(
            name="qkvp", bufs=1
        ) as qkvp, tc.tile_pool(name="v65", bufs=1) as v65p, tc.tile_pool(
            name="pp", bufs=4
        ) as ppool, tc.tile_pool(name="sml", bufs=4) as smlp, tc.tile_pool(
            name="ps_st", bufs=2, space=PSUM
        ) as stp, tc.tile_pool(name="ps_po", bufs=2, space=PSUM) as pop, tc.tile_pool(
            name="ps_mm", bufs=2, space=PSUM
        ) as mmp:
            # ---- constants ----
            ident = const.tile([128, 128], f32)
            make_identity(nc, ident)
            ident_b = const.tile([128, 128], bf16)
            nc.vector.tensor_copy(ident_b[:], ident[:])
            tri = const.tile([128, 128], f32)
            make_upper_triangular(nc, tri, val=1.0, diag=True)  # 1 where part<=free
            tri_b = const.tile([128, 128], bf16)
            nc.vector.tensor_copy(tri_b[:], tri[:])
            bias_sb = const.tile([128, MQKV // 128], f32)
            nc.sync.dma_start(bias_sb[:], bqkv_d.ap().rearrange("(m p) -> p m", p=128))
            # separate tiles per chunk so region deps let compute start as
            # soon as the first chunks land; issue order: w[k0:2], xt[nb0],
            # remaining w, remaining xt, wp
            w_sb = [const.tile([128, 2, MQKV], bf16, name=f"w_{kc}") for kc in range(4)]
            xt = [const.tile([128, KT, 512], bf16, name=f"xt_{nb}") for nb in range(NBB)]
            nc.sync.dma_start(w_sb[0][:], w_src[:, 0:2, :])
            nc.sync.dma_start(xt[0][:], xT_src[:, :, 0:512])
            for kc in range(1, 4):
                nc.sync.dma_start(w_sb[kc][:], w_src[:, 2 * kc : 2 * kc + 2, :])
            for nb in range(1, NBB):
                nc.sync.dma_start(
                    xt[nb][:], xT_src[:, :, 512 * nb : 512 * (nb + 1)]
                )
            wp_sb = const.tile([128, NPAIR, C], bf16)
            nc.sync.dma_start(wp_sb[:], wp_src)

            # v65[pair]: [128 keys, key-tile i, head-in-pair hh, 64 dims + ones]
            v65 = [
                const.tile([128, TTK, 2, HS + 1], bf16, name=f"v65_{p}")
                for p in range(NPAIR)
            ]
            for p in range(NPAIR):
                nc.vector.memset(v65[p][:, :, :, HS : HS + 1], 1.0)

            # attn_pack[pair]: [128 = 2*HS head dims, T] bf16 proj input
            attn_pack = [
                const.tile([128, T], bf16, name=f"attn_pack_{p}") for p in range(NPAIR)
            ]

            # ---- phase 1: qkv^T (m-tile m covers 128 qkv columns) ----
            qkvT = [
                qkvp.tile([128, T], bf16, tag=f"qkvT{m}", name=f"qkvT{m}")
                for m in range(MQKV // 128)
            ]
            for nb in range(NBB):
                for m in range(MQKV // 128):
                    pq = mmp.tile([128, 512], f32, tag="mm")
                    for k in range(KT):
                        nc.tensor.matmul(
                            pq[:],
                            w_sb[k // 2][:, k % 2, 128 * m : 128 * (m + 1)],
                            xt[nb][:, k, :],
                            start=(k == 0),
                            stop=(k == KT - 1),
                        )
                    # bias + cast on ACT (idle during this phase)
                    nc.scalar.activation(
                        qkvT[m][:, 512 * nb : 512 * (nb + 1)],
                        pq[:],
                        IDENTF,
                        bias=bias_sb[:, m : m + 1],
                    )

            # ---- phase 1.5: v transposes, 2 heads per [128,128] tile ----
            # emitted lazily inside the first tqb's pair loop so pairs 1-3
            # transpose while pair 0's attention already feeds ACT
            def emit_vt(p):
                vm = qkvT[2 * NPAIR + p]  # v m-tile of this pair
                for i in range(TTK):
                    pt = mmp.tile([128, 128], bf16, tag="mm", name="vt")
                    nc.tensor.transpose(
                        pt[:], vm[:, 128 * i : 128 * (i + 1)], ident_b[:]
                    )
                    # psum [128 keys, 2*64 dims] -> v65[:, i, hh, 0:64]
                    nc.vector.tensor_copy(v65[p][:, i, :, 0:HS], pt[:])

            # ---- phase 2+3: attention (tqb descending) + proj per tqb ----
            def emit_proj(tqb):
                for t in range(4 * tqb, 4 * tqb + 4):
                    ty = smlp.tile([128, C], f32, tag="y", bufs=3, name="ty")
                    for n in range(2):
                        py = mmp.tile([128, 512], f32, tag="mm", name="py")
                        for p in range(NPAIR):
                            nc.tensor.matmul(
                                py[:],
                                attn_pack[p][:, 128 * t : 128 * (t + 1)],
                                wp_sb[:, p, 512 * n : 512 * (n + 1)],
                                start=(p == 0),
                                stop=(p == NPAIR - 1),
                            )
                        if n == 0:
                            nc.vector.tensor_copy(
                                ty[:, 512 * n : 512 * (n + 1)], py[:]
                            )
                        else:
                            nc.scalar.copy(ty[:, 512 * n : 512 * (n + 1)], py[:])
                    nc.scalar.dma_start(y_d[128 * t : 128 * (t + 1), :], ty[:])

            pending_proj = None
            for tqbi in range(NBB):
                tqb = NBB - 1 - tqbi
                q0 = 512 * tqb
                ntk = 4 * (tqb + 1)
                for p in range(NPAIR):
                    if tqbi == 0:
                        emit_vt(p)
                    qm = qkvT[p]
                    km = qkvT[NPAIR + p]
                    for hh in range(2):
                        hp = HS * hh
                        po = pop.tile([HS + 1, 512], f32, tag="po")
                        for ig in range(ntk // 2):
                            st = stp.tile([128, 1024], f32, tag="st")
                            ptile = ppool.tile([128, 1024], bf16, tag="p", name="pt")
                            # pack tile u=1 right after u=0's end so one exp
                            # call covers [vf0 : 1024-vf1] contiguously
                            i0, i1 = 2 * ig, 2 * ig + 1
                            vf0 = max(0, 128 * i0 - q0)
                            vf1 = max(0, 128 * i1 - q0)
                            nc.tensor.matmul(
                                st[:, vf0:512],
                                km[hp : hp + HS, 128 * i0 : 128 * (i0 + 1)],
                                qm[hp : hp + HS, q0 + vf0 : q0 + 512],
                                start=True,
                                stop=True,
                            )
                            nc.tensor.matmul(
                                st[:, 512 : 1024 - vf1],
                                km[hp : hp + HS, 128 * i1 : 128 * (i1 + 1)],
                                qm[hp : hp + HS, q0 + vf1 : q0 + 512],
                                start=True,
                                stop=True,
                            )
                            nc.scalar.activation(
                                ptile[:, vf0 : 1024 - vf1],
                                st[:, vf0 : 1024 - vf1],
                                EXPF,
                            )
                            if 128 * i0 >= q0:  # diagonal tiles: causal mask
                                nc.vector.tensor_mul(
                                    ptile[:, vf0 : vf0 + 128],
                                    ptile[:, vf0 : vf0 + 128],
                                    tri_b[:],
                                )
                            if 128 * i1 >= q0:
                                nc.vector.tensor_mul(
                                    ptile[:, 512:640],
                                    ptile[:, 512:640],
                                    tri_b[:],
                                )
                            nc.tensor.matmul(
                                po[:, vf0:512],
                                v65[p][:, i0, hh, :],
                                ptile[:, vf0:512],
                                start=(i0 == 0),
                                stop=False,
                            )
                            nc.tensor.matmul(
                                po[:, vf1:512],
                                v65[p][:, i1, hh, :],
                                ptile[:, 512 : 1024 - vf1],
                                start=False,
                                stop=(i1 == ntk - 1),
                            )
                        # custom-DVE ops ignore the input partition base, so
                        # hop the denominator row to partition 0 first
                        zrow = smlp.tile([1, 512], f32, tag="zrow")
                        nc.vector.tensor_copy(zrow[:], po[HS : HS + 1, :])
                        recip = smlp.tile([1, 512], f32, tag="rcp")
                        nc.vector.reciprocal_approx_fast(recip[:], zrow[:])
                        rb = smlp.tile([HS, 512], f32, tag="rb")
                        nc.gpsimd.partition_broadcast(rb[:], recip[:])
                        if hh == 0:
                            nc.vector.tensor_mul(
                                attn_pack[p][0:HS, q0 : q0 + 512],
                                po[0:HS, :],
                                rb[:],
                            )
                        else:
                            a1 = smlp.tile([HS, 512], bf16, tag="a1")
                            nc.vector.tensor_mul(a1[:], po[0:HS, :], rb[:])
                            nc.sync.dma_start(
                                attn_pack[p][HS:128, q0 : q0 + 512], a1[:]
                            )
                    # interleave the previous tqb's proj behind pair 0 so the
                    # PE's proj work doesn't starve ACT at the tqb boundary
                    if p == 0 and pending_proj is not None:
                        emit_proj(pending_proj)
                        pending_proj = None
                pending_proj = tqb
            emit_proj(pending_proj)

    nc.compile()
    return nc


_NC_CACHE = None


def _get_nc():
    global _NC_CACHE
    if _NC_CACHE is None:
        _NC_CACHE = build_nc()
    return _NC_CACHE


def make_in_maps(x, w_attn, b_attn, w_proj):
    x = np.asarray(x, np.float32).reshape(B, T, C)
    w_attn = np.asarray(w_attn, np.float32)
    b_attn = np.asarray(b_attn, np.float32)
    w_proj = np.asarray(w_proj, np.float32)
    scale = 1.0 / np.sqrt(HS)
    in_maps = []
    for c in range(NCORES):
        b, j = divmod(c, 2)
        cs = slice(HS * HPC * j, HS * HPC * (j + 1))
        wq = w_attn[:, 0 * C :][:, cs] * scale
        wk = w_attn[:, 1 * C : 2 * C][:, cs]
        wv = w_attn[:, 2 * C : 3 * C][:, cs]
        wqkv = np.concatenate([wq, wk, wv], axis=1)
        bq = b_attn[0 * C :][cs] * scale
        bk = b_attn[1 * C : 2 * C][cs]
        bv = b_attn[2 * C : 3 * C][cs]
        bqkv = np.ascontiguousarray(np.concatenate([bq, bk, bv]))
        in_maps.append(
            {
                "xT": np.ascontiguousarray(x[b].T).astype(ml_dtypes.bfloat16),
                "wqkv": wqkv.astype(ml_dtypes.bfloat16),
                "bqkv": bqkv,
                "wp": w_proj[cs, :].astype(ml_dtypes.bfloat16),
            }
        )
    return in_maps


def run_on_device(in_maps, **kwargs):
    nc = _get_nc()
    return run_bass_kernel_spmd(nc, in_maps, core_ids=list(range(NCORES)), **kwargs)


def kernel(x, w_attn, b_attn, w_proj, b_proj):
    in_maps = make_in_maps(x, w_attn, b_attn, w_proj)
    res = run_on_device(in_maps)
    b_proj = np.asarray(b_proj, np.float32)
    y = np.empty((B, T, C), np.float32)
    for b in range(B):
        y[b] = res.results[2 * b]["y"]
        y[b] += res.results[2 * b + 1]["y"]
        y[b] += b_proj
    return y


if __name__ == "__main__":
    rng = np.random.default_rng(0)
    x = rng.standard_normal((B, T, C)).astype(np.float32)
    w_attn = (rng.standard_normal((C, 3 * C)) * 0.02).astype(np.float32)
    b_attn = np.zeros(3 * C, np.float32)
    w_proj = (rng.standard_normal((C, C)) * 0.02).astype(np.float32)
    b_proj = np.zeros(C, np.float32)
    y = kernel(x, w_attn, b_attn, w_proj, b_proj)
    print("out", y.shape, y.dtype, y[0, 0, :4])


# revision 14
# speedup vs baseline: 2.0335x; 1.0032x over previous
"""Causal self-attention (B=4, T=2048, C=1024, H=16) on 8 TRN2 NeuronCores.

Sharding: batch x head-half. Core c handles batch c//2 and heads
8*(c%2) .. 8*(c%2)+8. Each core computes qkv for its 8 heads (w_attn column
shard), full causal attention for those heads, and a partial c_proj product
(w_proj row shard). The host sums the two partials per batch (the 2-way
all-reduce of the TP scheme) and adds b_proj.

Per-core program (S^T formulation: softmax reduces on the free axis via an
appended ones-column in the PV weights; no max-subtraction, scores are
bounded ~N(0, 0.41^2)):
  phase 1: qkv^T = wqkv^T @ x^T (bf16 matmuls, f32 psum); ACT engine applies
    bias and casts psum->sbuf bf16 (it is otherwise idle here).
  phase 1.5: v^T tiles are PE-transposed pairwise ([128,128] covers 2 heads)
    into v65 tiles [128, 65] bf16 whose col 64 is ones (denominator trick).
  phase 2: for tqb (512-query blocks) DESCENDING, for each head: S^T tiles
    k_i^T-stationary x q^T-moving into [128,1024] psum groups (2 key tiles),
    one ACT exp per group -> P bf16; causal diag tiles masked by a DVE
    triangular multiply; PV accumulates [65,512] per query block; the
    denominator row is inverted with reciprocal_approx_fast (DVE),
    partition-broadcast on GpSimd, and multiplied into attn out (DVE).
  phase 3 (interleaved per tqb): proj tiles y = attn_pack @ wp accumulate
    over the 4 head-pairs in psum, copy to sbuf (DVE/ACT alternating), and
    DMA to DRAM. Proj for query block tqb is emitted behind pair 0 of the
    next (descending) tqb so the PE's proj work never starves ACT.
"""

import sys

for _p in (
    "/opt/trn_rl_repo",
    "/root/.axon_site/_ro/trn_rl_repo",
):
    if _p not in sys.path:
        sys.path.append(_p)

import numpy as np
import ml_dtypes
import concourse.bacc as bacc
import concourse.mybir as mybir
import concourse.tile as tile
from concourse.bass_utils import run_bass_kernel_spmd
from concourse.masks import make_identity, make_upper_triangular

B, T, C, H = 4, 2048, 1024, 16
HS = C // H           # 64
NCORES = 8
HPC = H // 2          # heads per core = 8
NPAIR = HPC // 2      # head pairs per core = 4
MQKV = 3 * HPC * HS   # 1536 qkv columns per core
KT = C // 128         # 8 contraction tiles
NBB = T // 512        # 4 query/token blocks
TTK = T // 128        # 16 key tiles per sequence
NT = T // 128         # 16 proj row tiles

f32 = mybir.dt.float32
bf16 = mybir.dt.bfloat16
EXPF = mybir.ActivationFunctionType.Exp
IDENTF = mybir.ActivationFunctionType.Identity
PSUM = "PSUM"


def build_nc():
    nc = bacc.Bacc("TRN2", target_bir_lowering=False, debug=False, num_devices=NCORES)
    xT_d = nc.dram_tensor("xT", [C, T], bf16, kind="ExternalInput")
    wqkv_d = nc.dram_tensor("wqkv", [C, MQKV], bf16, kind="ExternalInput")
    bqkv_d = nc.dram_tensor("bqkv", [MQKV], f32, kind="ExternalInput")
    wp_d = nc.dram_tensor("wp", [HPC * HS, C], bf16, kind="ExternalInput")
    y_d = nc.dram_tensor("y", [T, C], f32, kind="ExternalOutput")

    w_src = wqkv_d.ap().rearrange("(k p) m -> p k m", p=128)
    xT_src = xT_d.ap().rearrange("(k p) n -> p k n", p=128)
    wp_src = wp_d.ap().rearrange("(k p) n -> p k n", p=128)

    with tile.TileContext(nc) as tc:
        with tc.tile_pool(name="const", bufs=1) as const, tc.tile_pool(
            name="qkvp", bufs=1
        ) as qkvp, tc.tile_pool(name="v65", bufs=1) as v65p, tc.tile_pool(
            name="pp", bufs=4
        ) as ppool, tc.tile_pool(name="sml", bufs=4) as smlp, tc.tile_pool(
            name="ps_st", bufs=2, space=PSUM
        ) as stp, tc.tile_pool(name="ps_po", bufs=2, space=PSUM) as pop, tc.tile_pool(
            name="ps_mm", bufs=2, space=PSUM
        ) as mmp:
            # ---- constants ----
            ident = const.tile([128, 128], f32)
            make_identity(nc, ident)
            ident_b = const.tile([128, 128], bf16)
            nc.vector.tensor_copy(ident_b[:], ident[:])
            tri = const.tile([128, 128], f32)
            make_upper_triangular(nc, tri, val=1.0, diag=True)  # 1 where part<=free
            tri_b = const.tile([128, 128], bf16)
            nc.vector.tensor_copy(tri_b[:], tri[:])
            bias_sb = const.tile([128, MQKV // 128], f32)
            nc.sync.dma_start(bias_sb[:], bqkv_d.ap().rearrange("(m p) -> p m", p=128))
            # separate tiles per chunk so region deps let compute start as
            # soon as the first chunks land; issue order: w[k0:2], xt[nb0],
            # remaining w, remaining xt, wp
            w_sb = [const.tile([128, 2, MQKV], bf16, name=f"w_{kc}") for kc in range(4)]
            xt = [const.tile([128, KT, 512], bf16, name=f"xt_{nb}") for nb in range(NBB)]
            nc.sync.dma_start(w_sb[0][:], w_src[:, 0:2, :])
            nc.sync.dma_start(xt[0][:], xT_src[:, :, 0:512])
            for kc in range(1, 4):
                nc.sync.dma_start(w_sb[kc][:], w_src[:, 2 * kc : 2 * kc + 2, :])
            for nb in range(1, NBB):
                nc.sync.dma_start(
                    xt[nb][:], xT_src[:, :, 512 * nb : 512 * (nb + 1)]
                )
            wp_sb = const.tile([128, NPAIR, C], bf16)
            nc.sync.dma_start(wp_sb[:], wp_src)

            # v65[pair]: [128 keys, key-tile i, head-in-pair hh, 64 dims + ones]
            v65 = [
                const.tile([128, TTK, 2, HS + 1], bf16, name=f"v65_{p}")
                for p in range(NPAIR)
            ]
            for p in range(NPAIR):
                nc.vector.memset(v65[p][:, :, :, HS : HS + 1], 1.0)

            # attn_pack[pair]: [128 = 2*HS head dims, T] bf16 proj input
            attn_pack = [
                const.tile([128, T], bf16, name=f"attn_pack_{p}") for p in range(NPAIR)
            ]

            # ---- phase 1: qkv^T (m-tile m covers 128 qkv columns) ----
            qkvT = [
                qkvp.tile([128, T], bf16, tag=f"qkvT{m}", name=f"qkvT{m}")
                for m in range(MQKV // 128)
            ]
            for nb in range(NBB):
                for m in range(MQKV // 128):
                    pq = mmp.tile([128, 512], f32, tag="mm")
                    for k in range(KT):
                        nc.tensor.matmul(
                            pq[:],
                            w_sb[k // 2][:, k % 2, 128 * m : 128 * (m + 1)],
                            xt[nb][:, k, :],
                            start=(k == 0),
                            stop=(k == KT - 1),
                        )
                    # bias + cast on ACT (idle during this phase)
                    nc.scalar.activation(
                        qkvT[m][:, 512 * nb : 512 * (nb + 1)],
                        pq[:],
                        IDENTF,
                        bias=bias_sb[:, m : m + 1],
                    )

            # ---- phase 1.5: v transposes, 2 heads per [128,128] tile ----
            # emitted lazily inside the first tqb's pair loop so pairs 1-3
            # transpose while pair 0's attention already feeds ACT
            def emit_vt(p):
                vm = qkvT[2 * NPAIR + p]  # v m-tile of this pair
                for i in range(TTK):
                    pt = mmp.tile([128, 128], bf16, tag="mm", name="vt")
                    nc.tensor.transpose(
                        pt[:], vm[:, 128 * i : 128 * (i + 1)], ident_b[:]
                    )
                    # psum [128 keys, 2*64 dims] -> v65[:, i, hh, 0:64]
                    nc.vector.tensor_copy(v65[p][:, i, :, 0:HS], pt[:])

            # ---- phase 2+3: attention (tqb descending) + proj per tqb ----
            def emit_proj(tqb):
                for t in range(4 * tqb, 4 * tqb + 4):
                    ty = smlp.tile([128, C], f32, tag="y", bufs=3, name="ty")
                    for n in range(2):
                        py = mmp.tile([128, 512], f32, tag="mm", name="py")
                        for p in range(NPAIR):
                            nc.tensor.matmul(
                                py[:],
                                attn_pack[p][:, 128 * t : 128 * (t + 1)],
                                wp_sb[:, p, 512 * n : 512 * (n + 1)],
                                start=(p == 0),
                                stop=(p == NPAIR - 1),
                            )
                        if n == 0:
                            nc.vector.tensor_copy(
                                ty[:, 512 * n : 512 * (n + 1)], py[:]
                            )
                        else:
                            nc.scalar.copy(ty[:, 512 * n : 512 * (n + 1)], py[:])
                    nc.scalar.dma_start(y_d[128 * t : 128 * (t + 1), :], ty[:])

            pending_proj = None
            for tqbi in range(NBB):
                tqb = NBB - 1 - tqbi
                q0 = 512 * tqb
                ntk = 4 * (tqb + 1)
                for p in range(NPAIR):
                    if tqbi == 0:
                        emit_vt(p)
                    qm = qkvT[p]
                    km = qkvT[NPAIR + p]
                    for hh in range(2):
                        hp = HS * hh
                        po = pop.tile([HS + 1, 512], f32, tag="po")
                        for ig in range(ntk // 2):
                            st = stp.tile([128, 1024], f32, tag="st")
                            ptile = ppool.tile([128, 1024], bf16, tag="p", name="pt")
                            # pack tile u=1 right after u=0's end so one exp
                            # call covers [vf0 : 1024-vf1] contiguously
                            i0, i1 = 2 * ig, 2 * ig + 1
                            vf0 = max(0, 128 * i0 - q0)
                            vf1 = max(0, 128 * i1 - q0)
                            nc.tensor.matmul(
                                st[:, vf0:512],
                                km[hp : hp + HS, 128 * i0 : 128 * (i0 + 1)],
                                qm[hp : hp + HS, q0 + vf0 : q0 + 512],
                                start=True,
                                stop=True,
                            )
                            nc.tensor.matmul(
                                st[:, 512 : 1024 - vf1],
                                km[hp : hp + HS, 128 * i1 : 128 * (i1 + 1)],
                                qm[hp : hp + HS, q0 + vf1 : q0 + 512],
                                start=True,
                                stop=True,
                            )
                            nc.scalar.activation(
                                ptile[:, vf0 : 1024 - vf1],
                                st[:, vf0 : 1024 - vf1],
                                EXPF,
                            )
                            if 128 * i0 >= q0:  # diagonal tiles: causal mask
                                nc.vector.tensor_mul(
                                    ptile[:, vf0 : vf0 + 128],
                                    ptile[:, vf0 : vf0 + 128],
                                    tri_b[:],
                                )
                            if 128 * i1 >= q0:
                                nc.vector.tensor_mul(
                                    ptile[:, 512:640],
                                    ptile[:, 512:640],
                                    tri_b[:],
                                )
                            nc.tensor.matmul(
                                po[:, vf0:512],
                                v65[p][:, i0, hh, :],
                                ptile[:, vf0:512],
                                start=(i0 == 0),
                                stop=False,
                            )
                            nc.tensor.matmul(
                                po[:, vf1:512],
                                v65[p][:, i1, hh, :],
                                ptile[:, 512 : 1024 - vf1],
                                start=False,
                                stop=(i1 == ntk - 1),
                            )
                        # custom-DVE ops ignore the input partition base, so
                        # hop the denominator row to partition 0 first
                        zrow = smlp.tile([1, 512], f32, tag="zrow")
                        nc.vector.tensor_copy(zrow[:], po[HS : HS + 1, :])
                        recip = smlp.tile([1, 512], f32, tag="rcp")
                        nc.vector.reciprocal_approx_fast(recip[:], zrow[:])
                        rb = smlp.tile([HS, 512], f32, tag="rb")
                        nc.gpsimd.partition_broadcast(rb[:], recip[:])
                        if hh == 0:
                            nc.vector.tensor_mul(
                                attn_pack[p][0:HS, q0 : q0 + 512],
                                po[0:HS, :],
                                rb[:],
                            )
                        else:
                            a1 = smlp.tile([HS, 512], bf16, tag="a1")
                            nc.vector.tensor_mul(a1[:], po[0:HS, :], rb[:])
                            nc.sync.dma_start(
                                attn_pack[p][HS:128, q0 : q0 + 512], a1[:]
                            )
                    # interleave the previous tqb's proj behind pair 0 so the
                    # PE's proj work doesn't starve ACT at the tqb boundary
                    if p == 0 and pending_proj is not None:
                        emit_proj(pending_proj)
                        pending_proj = None
                pending_proj = tqb
            emit_proj(pending_proj)

    nc.compile()
    return nc


_NC_CACHE = None


def _get_nc():
    global _NC_CACHE
    if _NC_CACHE is None:
        _NC_CACHE = build_nc()
    return _NC_CACHE


def make_in_maps(x, w_attn, b_attn, w_proj):
    x = np.asarray(x, np.float32).reshape(B, T, C)
    w_attn = np.asarray(w_attn, np.float32)
    b_attn = np.asarray(b_attn, np.float32)
    w_proj = np.asarray(w_proj, np.float32)
    scale = 1.0 / np.sqrt(HS)
    in_maps = []
    for c in range(NCORES):
        b, j = divmod(c, 2)
        cs = slice(HS * HPC * j, HS * HPC * (j + 1))
        wq = w_attn[:, 0 * C :][:, cs] * scale
        wk = w_attn[:, 1 * C : 2 * C][:, cs]
        wv = w_attn[:, 2 * C : 3 * C][:, cs]
        wqkv = np.concatenate([wq, wk, wv], axis=1)
        bq = b_attn[0 * C :][cs] * scale
        bk = b_attn[1 * C : 2 * C][cs]
        bv = b_attn[2 * C : 3 * C][cs]
        bqkv = np.ascontiguousarray(np.concatenate([bq, bk, bv]))
        in_maps.append(
            {
                "xT": np.ascontiguousarray(x[b].T).astype(ml_dtypes.bfloat16),
                "wqkv": wqkv.astype(ml_dtypes.bfloat16),
                "bqkv": bqkv,
                "wp": w_proj[cs, :].astype(ml_dtypes.bfloat16),
            }
        )
    return in_maps


def run_on_device(in_maps, **kwargs):
    nc = _get_nc()
    return run_bass_kernel_spmd(nc, in_maps, core_ids=list(range(NCORES)), **kwargs)


def kernel(x, w_attn, b_attn, w_proj, b_proj):
    in_maps = make_in_maps(x, w_attn, b_attn, w_proj)
    res = run_on_device(in_maps)
    b_proj = np.asarray(b_proj, np.float32)
    y = np.empty((B, T, C), np.float32)
    for b in range(B):
        y[b] = res.results[2 * b]["y"]
        y[b] += res.results[2 * b + 1]["y"]
        y[b] += b_proj
    return y


if __name__ == "__main__":
    rng = np.random.default_rng(0)
    x = rng.standard_normal((B, T, C)).astype(np.float32)
    w_attn = (rng.standard_normal((C, 3 * C)) * 0.02).astype(np.float32)
    b_attn = np.zeros(3 * C, np.float32)
    w_proj = (rng.standard_normal((C, C)) * 0.02).astype(np.float32)
    b_proj = np.zeros(C, np.float32)
    y = kernel(x, w_attn, b_attn, w_proj, b_proj)
    print("out", y.shape, y.dtype, y[0, 0, :4])
